# revision 5
# baseline (speedup 1.0000x reference)
"""Trainium2 Bass kernel v3: quad-table gather for K-Planes lookup + MLP.

Key idea: the baseline gathers one 256B delta-form row per
(point, scale, plane) = 12 dma_gather indices per point, and the Q7
SWDGE descriptor generation (~8ns/index) dominates (90% of runtime).

v3 restructures the table so ONE gathered row serves all 4 scales of
one plane: row (iy, pl, j2) holds 12 corner COLUMNS [v(16ch), dy(16ch)]
(y-lerp delta form):
    s0: cols e0,e0+1,e0+2   e0 = (j2-3)//4
    s1: cols e1,e1+1,e1+2   e1 = (j2-1)//2
    s2: cols j2,j2+1
    s3: cols 2j2..2j2+3
where j2 = s2 cell of the point.  The candidate sets provably cover the
cells every scale needs.  x-interp becomes a hat-weighted sum over the
columns: res_s = sum_m relu(1-|z_s-m|) * (v_m + wy*dy_m), z_s = local
fractional coordinate (host-computed).  3 indices/point instead of 12.

bf16 table + bf16 interp + bf16 MLP (rel err ~7e-3 vs 2e-2 budget).
Host precomputes idx16 (wrapped+replicated int16) and the 13-stream
z/wy weights; the device does zero index math.
"""

import math
import numpy as np
import ml_dtypes
from contextlib import ExitStack

import concourse.bass as bass
import concourse.bacc as bacc
import concourse.mybir as mybir
import concourse.tile as tile
from concourse import library_config
from concourse.masks import make_identity

FP = mybir.dt.float32
BF = mybir.dt.bfloat16
I16 = mybir.dt.int16

H = 150
WS = (64, 128, 256, 512)
NP = 3
NS = 4
NCORES = 8
YB = 42
NBKT = (H - 2) // YB + 1          # iy0 in [0,148] -> 5 buckets
J2N = WS[2] - 1                   # 255 j2 values (cells 0..254 + clamp pad)
RPY = NP * J2N                    # rows per iy = 765
NCOL = 12                         # corner columns per row
RW = NCOL * 32                    # row elements (bf16): 384 = 768B
SLOT_S = [0, 0, 0, 1, 1, 1, 2, 2, 3, 3, 3, 3]   # scale of each col slot
SLOT_M = [0, 1, 2, 0, 1, 2, 0, 1, 0, 1, 2, 3]   # m offset of each col slot
SLOT_OFF = [0, 3, 6, 8]                          # first slot of each scale
SLOT_N = [3, 3, 2, 4]                            # cols per scale

K = 16            # point-cols per partition per block (block = 2048 pts)
MM_N = 512


def win_lo(b):
    return b * YB


def win_rows(b):
    return min(H - 1, (b + 1) * YB) - win_lo(b)   # <= 32


# ---------------------------------------------------------------------------
# device program
# ---------------------------------------------------------------------------

def build_program(block_buckets, k: int = K, num_devices: int = 1):
    nc = bacc.Bacc("TRN2", target_bir_lowering=False, debug=False,
                   enable_asserts=False, num_devices=num_devices)

    KC = k
    nb = len(block_buckets)
    L = nb * 128 * KC
    NIDX = NP * KC * 128

    wh_d = nc.dram_tensor("wh", [nb, 128, NP * KC * NCOL * 2], BF,
                          kind="ExternalInput").ap()
    idx_d = nc.dram_tensor("idx16", [nb, 128, NP * KC * 8], I16,
                           kind="ExternalInput").ap()
    tab_d = nc.dram_tensor("tab", [(H - 1) * RPY, RW], BF, kind="ExternalInput").ap()
    w0t_d = nc.dram_tensor("w0t", [64, 128], BF, kind="ExternalInput").ap()
    w1t_d = nc.dram_tensor("w1t", [128, 128], BF, kind="ExternalInput").ap()
    w2t_d = nc.dram_tensor("w2t", [128, 64], BF, kind="ExternalInput").ap()
    b0_d = nc.dram_tensor("b0c", [128, 1], FP, kind="ExternalInput").ap()
    b1_d = nc.dram_tensor("b1c", [128, 1], FP, kind="ExternalInput").ap()
    b2_d = nc.dram_tensor("b2r", [128, 64], FP, kind="ExternalInput").ap()
    out_d = nc.dram_tensor("out", [L, 64], FP, kind="ExternalOutput").ap()

    with tile.TileContext(nc) as tc:
        with ExitStack() as ctx:
            cpool = ctx.enter_context(tc.tile_pool(name="cpool", bufs=1))
            ppool = ctx.enter_context(tc.tile_pool(name="ppool", bufs=3))
            gpool = ctx.enter_context(tc.tile_pool(name="gpool", bufs=3))
            ipool = ctx.enter_context(tc.tile_pool(name="ipool", bufs=2))
            fpool = ctx.enter_context(tc.tile_pool(name="fpool", bufs=2))
            mpool = ctx.enter_context(tc.tile_pool(name="mpool", bufs=2))
            qpool = ctx.enter_context(tc.tile_pool(name="qpool", bufs=2, space="PSUM"))

            nc.gpsimd.load_library(library_config.mlp)

            identf = cpool.tile([128, 128], FP)
            make_identity(nc, identf)
            ident = cpool.tile([128, 128], BF)
            nc.vector.tensor_copy(ident, identf)
            w0t = cpool.tile([64, 128], BF)
            nc.sync.dma_start(w0t, w0t_d)
            w1t = cpool.tile([128, 128], BF)
            nc.sync.dma_start(w1t, w1t_d)
            w2t = cpool.tile([128, 64], BF)
            nc.sync.dma_start(w2t, w2t_d)
            b0 = cpool.tile([128, 1], FP)
            nc.sync.dma_start(b0, b0_d)
            b1 = cpool.tile([128, 1], FP)
            nc.sync.dma_start(b1, b1_d)
            b2r = cpool.tile([128, 64], FP)
            nc.sync.dma_start(b2r, b2_d)

            for blk in range(nb):
                bkt = block_buckets[blk]
                wh = ppool.tile([128, NP * KC * NCOL * 2], BF)
                nc.sync.dma_start(wh, wh_d[blk])
                idx16 = ppool.tile([128, NP * KC * 8], I16)
                nc.sync.dma_start(idx16, idx_d[blk])

                # ---- gather: one row per (pt, plane) ----
                base = win_lo(bkt) * RPY
                wrows = win_rows(bkt) * RPY
                g = gpool.tile([128, NP * KC * RW], BF, tag="g")
                nc.gpsimd.dma_gather(
                    out_ap=g.rearrange("p (c i) -> p c i", i=RW),
                    in_ap=tab_d[base:base + wrows],
                    idxs_ap=idx16,
                    num_idxs=NIDX,
                    num_idxs_reg=NIDX,
                    elem_size=RW,
                    single_packet=False)

                # ---- apply: g *= wh (broadcast over 16 channels) ----
                gv = g.rearrange("p (pl c sh ch) -> p pl c sh ch",
                                 pl=NP, c=KC, ch=16)
                whb = (wh.rearrange("p (pl c sh) -> p pl c sh", pl=NP, c=KC)
                       .unsqueeze(-1).to_broadcast([128, NP, KC, NCOL * 2, 16]))
                nc.vector.tensor_tensor(out=gv, in0=gv, in1=whb,
                                        op=mybir.AluOpType.mult)
                # fold v/dy halves: tcol = g[..., v] + g[..., dy]
                tcol = ipool.tile([128, NP * KC * NCOL * 16], BF)
                tv = tcol.rearrange("p (pl c sl ch) -> p pl c sl ch",
                                    pl=NP, c=KC, ch=16)
                g6 = g.rearrange("p (pl c sl h ch) -> p pl c sl h ch",
                                 pl=NP, c=KC, h=2, ch=16)
                nc.vector.tensor_tensor(out=tv, in0=g6[:, :, :, :, 0, :],
                                        in1=g6[:, :, :, :, 1, :],
                                        op=mybir.AluOpType.add)
                # ragged adds per scale -> res[p, pl, c, 64]
                res = ipool.tile([128, NP * KC * 64], BF)
                rv = res.rearrange("p (pl c f) -> p pl c f", pl=NP, c=KC)
                for s in range(NS):
                    o = SLOT_OFF[s]
                    dst = rv[:, :, :, s * 16:(s + 1) * 16]
                    nc.vector.tensor_tensor(
                        out=dst, in0=tv[:, :, :, o, :], in1=tv[:, :, :, o + 1, :],
                        op=mybir.AluOpType.add)
                    for m in range(2, SLOT_N[s]):
                        nc.vector.tensor_tensor(
                            out=dst, in0=dst, in1=tv[:, :, :, o + m, :],
                            op=mybir.AluOpType.add)
                # plane product -> feats [p, c, 64]
                pp = ipool.tile([128, KC * 64], BF)
                ppv = pp.rearrange("p (c f) -> p c f", f=64)
                nc.vector.tensor_tensor(out=ppv, in0=rv[:, 0],
                                        in1=rv[:, 1],
                                        op=mybir.AluOpType.mult)
                feats = fpool.tile([128, KC * 64], BF)
                nc.vector.tensor_tensor(out=feats.rearrange("p (c f) -> p c f", f=64),
                                        in0=ppv,
                                        in1=rv[:, 2],
                                        op=mybir.AluOpType.mult)
                featsv = feats.rearrange("p (c i) -> p c i", i=64)

                # ---- MLP (bf16 matmuls, fp32 psum) ----
                outt = fpool.tile([128, KC * 64], FP)
                nchunk = (KC * 128) // MM_N
                kper = MM_N // 128
                for cc in range(nchunk):
                    ftp = qpool.tile([64, MM_N], BF, space="PSUM", tag="ftp")
                    for j in range(kper):
                        kk = cc * kper + j
                        nc.tensor.transpose(
                            out=ftp[:, j * 128:(j + 1) * 128],
                            in_=featsv[:, kk, :], identity=ident)
                    fts = mpool.tile([64, MM_N], BF)
                    nc.scalar.activation(fts, ftp,
                                         mybir.ActivationFunctionType.Copy)
                    p0 = qpool.tile([128, MM_N], FP, space="PSUM", tag="p0")
                    nc.tensor.matmul(out=p0, lhsT=w0t, rhs=fts,
                                     start=True, stop=True)
                    h0 = mpool.tile([128, MM_N], BF)
                    nc.scalar.activation(h0, p0,
                                         mybir.ActivationFunctionType.Relu,
                                         bias=b0[:, 0:1])
                    p1 = qpool.tile([128, MM_N], FP, space="PSUM", tag="p1")
                    nc.tensor.matmul(out=p1, lhsT=w1t, rhs=h0,
                                     start=True, stop=True)
                    h1 = mpool.tile([128, MM_N], BF)
                    nc.scalar.activation(h1, p1,
                                         mybir.ActivationFunctionType.Relu,
                                         bias=b1[:, 0:1])
                    p2 = qpool.tile([128, kper * 64], FP, space="PSUM", tag="p2")
                    for j in range(kper):
                        nc.tensor.matmul(out=p2[:, j * 64:(j + 1) * 64],
                                         lhsT=h1[:, j * 128:(j + 1) * 128],
                                         rhs=w2t, start=True, stop=True)
                    for j in range(kper):
                        kk = cc * kper + j
                        nc.vector.tensor_tensor(
                            out=outt[:, kk * 64:(kk + 1) * 64],
                            in0=p2[:, j * 64:(j + 1) * 64], in1=b2r,
                            op=mybir.AluOpType.add)

                nc.sync.dma_start(
                    out_d[blk * 128 * KC:(blk + 1) * 128 * KC]
                    .rearrange("(p c) f -> p (c f)", p=128),
                    outt)

    nc.compile()
    return nc


# ---------------------------------------------------------------------------
# host-side data prep
# ---------------------------------------------------------------------------

def make_table(planes_list):
    """-> [(H-1)*765, 384] bf16; row (iy*3+pl)*255 + j2."""
    j2 = np.arange(J2N)
    e0 = (j2 - 3) // 4
    e1 = (j2 - 1) // 2
    cols = np.stack([e0, e0 + 1, e0 + 2, e1, e1 + 1, e1 + 2,
                     j2, j2 + 1, 2 * j2, 2 * j2 + 1, 2 * j2 + 2, 2 * j2 + 3],
                    axis=1)                                    # [255, 12]
    tab = np.empty((H - 1, NP, J2N, NCOL, 32), np.float32)
    for sl in range(NCOL):
        s = SLOT_S[sl]
        P = planes_list[s]                                     # [3,16,150,W]
        c = np.clip(cols[:, sl], 0, WS[s] - 1)                 # [255]
        v = P[:, :, :, c]                                      # [3,16,150,255]
        vt = v.transpose(0, 2, 3, 1)                           # [3,150,255,16]
        tab[:, :, :, sl, 0:16] = vt[:, :H - 1].transpose(1, 0, 2, 3)
        tab[:, :, :, sl, 16:32] = (vt[:, 1:] - vt[:, :H - 1]).transpose(1, 0, 2, 3)
    return np.ascontiguousarray(
        tab.reshape((H - 1) * RPY, RW)).astype(ml_dtypes.bfloat16)


def make_cns():
    c = np.zeros((128, NCOL), np.float32)
    c[:] = np.array(SLOT_M, np.float32)
    return c


def bucket_of_t(t):
    ay = np.float32(0.5 * (H - 1))
    iyf = np.clip(t.astype(np.float32) * ay + ay, 0.0, H - 1)
    iy0 = np.minimum(np.floor(iyf), H - 2).astype(np.int64)
    iy0 = np.maximum(iy0, 0)
    return iy0 // YB


def idx_weights(shard, block_buckets, k):
    """shard [L,4] -> wh [nb,128,3k*24] bf16, idx16 [nb,128,3k*8] int16.

    wh[b, p, pl, c, slot, 0] = hat  = relu(1 - |z_s - m|)
    wh[b, p, pl, c, slot, 1] = hat * wy
    """
    nb = len(block_buckets)
    arr = shard.reshape(nb, 128, k, 4)
    x = arr[..., 0:3]                                          # [nb,128,k,3]
    t = arr[..., 3]
    ay = np.float32(0.5 * (H - 1))
    iyf = np.clip(t * ay + ay, 0.0, H - 1)
    iy0 = np.minimum(np.floor(iyf), H - 2).astype(np.int64)
    wy = (iyf - iy0).astype(np.float32)                        # [nb,128,k]

    ixf = []
    for s in range(NS):
        a = np.float32(0.5 * (WS[s] - 1))
        ixf.append(np.clip(x * a + a, 0.0, WS[s] - 1))         # [nb,128,k,3]
    j2 = np.minimum(np.floor(ixf[2]), WS[2] - 2).astype(np.int64)
    e0 = (j2 - 3) // 4
    e1 = (j2 - 1) // 2
    zbase = [e0, e1, j2, 2 * j2]

    bf = ml_dtypes.bfloat16
    wh = np.empty((nb, 128, NP, k, NCOL, 2), bf)
    for s in range(NS):
        z = (ixf[s] - zbase[s]).astype(np.float32)             # [nb,128,k,3pl]
        for m in range(SLOT_N[s]):
            hat = np.maximum(0.0, 1.0 - np.abs(z - m)).astype(bf)
            hd = (hat.astype(np.float32) * wy[..., None]).astype(bf)
            sl = SLOT_OFF[s] + m
            wh[:, :, :, :, sl, 0] = hat.transpose(0, 1, 3, 2)
            wh[:, :, :, :, sl, 1] = hd.transpose(0, 1, 3, 2)
    wh = np.ascontiguousarray(wh.reshape(nb, 128, NP * k * NCOL * 2))

    wlo = np.array([win_lo(b) for b in block_buckets], np.int64)
    rowi = ((iy0[..., None] * NP + np.arange(NP)) * J2N + j2
            - (wlo[:, None, None, None] * RPY))                # [nb,128,k,3]
    assert rowi.min() >= 0 and rowi.max() < 32768, (rowi.min(), rowi.max())

    idx_r = rowi.transpose(0, 1, 3, 2).reshape(nb, 128, NP * k)
    w16 = idx_r.reshape(nb, 8, 16, NP * k).transpose(0, 2, 3, 1)
    w16 = w16.reshape(nb, 16, NP * k * 8)
    w16 = np.broadcast_to(w16[:, None], (nb, 8, 16, NP * k * 8))
    return wh, np.ascontiguousarray(
        w16.reshape(nb, 128, NP * k * 8)).astype(np.int16)


def bucket_layout(pts, k):
    """Globally balance points across cores per bucket (round-robin within
    each bucket) so per-core bucket counts differ by <=1 and block padding
    is minimal.  perm values are GLOBAL point indices."""
    n = pts.shape[0]
    pb = 128 * k
    bkt = bucket_of_t(pts[:, 3])
    order = np.argsort(bkt, kind="stable")          # global, bucket-sorted
    cores_idx = [[] for _ in range(NCORES)]
    nb_per_bucket = []
    pos = 0
    for b in range(NBKT):
        nb_in_b = int((bkt == b).sum())
        sel = order[pos:pos + nb_in_b]
        pos += nb_in_b
        per = [sel[c::NCORES] for c in range(NCORES)]
        mx = max(len(p) for p in per)
        nb_per_bucket.append(int(math.ceil(mx / pb)) if mx else 0)
        for c in range(NCORES):
            cores_idx[c].append(per[c])
    block_buckets = []
    for b in range(NBKT):
        block_buckets += [b] * nb_per_bucket[b]
    nb = len(block_buckets)
    L = nb * pb

    cores = []
    for c in range(NCORES):
        rows = np.zeros((L, 4), np.float32)
        perm = np.full(L, -1, np.int64)
        pos = 0
        for b in range(NBKT):
            sel = cores_idx[c][b]
            nrows = nb_per_bucket[b] * pb
            rows[pos:pos + len(sel)] = pts[sel]
            tpad = (b * YB + YB // 2) / (0.5 * (H - 1)) - 1.0
            if nrows > len(sel):
                rows[pos + len(sel):pos + nrows, 3] = tpad
            perm[pos:pos + len(sel)] = sel
            pos += nrows
        cores.append((rows, perm))
    return cores, block_buckets


def host_inputs(pts, planes_list, w0, b0, w1, b1, w2, b2, k=K):
    bf = ml_dtypes.bfloat16
    shared = {
        "tab": make_table(planes_list),
        "w0t": np.ascontiguousarray(w0.T).astype(bf),
        "w1t": np.ascontiguousarray(w1.T).astype(bf),
        "w2t": np.ascontiguousarray(w2.T).astype(bf),
        "b0c": np.ascontiguousarray(b0.reshape(128, 1)),
        "b1c": np.ascontiguousarray(b1.reshape(128, 1)),
        "b2r": np.ascontiguousarray(np.broadcast_to(b2.reshape(1, 64), (128, 64))),
    }
    cores, block_buckets = bucket_layout(pts, k)
    in_maps, perms = [], []
    for rows, perm in cores:
        wh, w16 = idx_weights(rows, block_buckets, k)
        in_maps.append({**shared, "wh": wh, "idx16": w16})
        perms.append(perm)
    return in_maps, perms, block_buckets


# ---------------------------------------------------------------------------
# numpy emulation (layout validation without HW)
# ---------------------------------------------------------------------------

def emulate(in_map, block_buckets, k=K):
    bf = ml_dtypes.bfloat16
    nb = len(block_buckets)
    whm = np.asarray(in_map["wh"]).reshape(nb, 128, NP, k, NCOL, 2)
    w16 = in_map["idx16"]
    tab = np.asarray(in_map["tab"], dtype=bf).astype(np.float32)
    out = np.empty((nb * 128 * k, 64), np.float32)
    for b in range(nb):
        base = win_lo(block_buckets[b]) * RPY
        idxs = w16[b, :16].reshape(16, NP * k, 8).astype(np.int64)
        rows_g = idxs.transpose(2, 0, 1).reshape(128, NP * k)
        g = tab[base + rows_g].reshape(128, NP, k, NCOL, 2, 16)
        whf = whm[b].astype(np.float32)
        feats = np.ones((128, k, 64), np.float32)
        for pl in range(NP):
            for s in range(NS):
                acc = np.zeros((128, k, 16), np.float32)
                for m in range(SLOT_N[s]):
                    sl = SLOT_OFF[s] + m
                    term = (g[:, pl, :, sl, 0] * whf[:, pl, :, sl, 0:1]
                            + g[:, pl, :, sl, 1] * whf[:, pl, :, sl, 1:2])
                    acc += term.astype(bf).astype(np.float32)
                feats[:, :, s * 16:(s + 1) * 16] *= acc.astype(bf).astype(np.float32)
        xx = feats.reshape(128 * k, 64)
        h = np.maximum(xx @ np.asarray(in_map["w0t"], dtype=bf).astype(np.float32), 0)
        h = np.maximum(h @ np.asarray(in_map["w1t"], dtype=bf).astype(np.float32), 0)
        y = h @ np.asarray(in_map["w2t"], dtype=bf).astype(np.float32) + in_map["b2r"][0]
        out[b * 128 * k:(b + 1) * 128 * k] = y
    return out


# ---------------------------------------------------------------------------
# entry point
# ---------------------------------------------------------------------------

_CACHE = {}


def kernel(pts, planes_s0, planes_s1, planes_s2, planes_s3,
           w0, b0, w1, b1, w2, b2, _want_trace=False):
    from concourse.bass_utils import run_bass_kernel_spmd

    pts = np.asarray(pts, np.float32)
    planes = [np.asarray(p, np.float32)
              for p in (planes_s0, planes_s1, planes_s2, planes_s3)]
    in_maps, perms, block_buckets = host_inputs(
        pts, planes,
        np.asarray(w0, np.float32), np.asarray(b0, np.float32),
        np.asarray(w1, np.float32), np.asarray(b1, np.float32),
        np.asarray(w2, np.float32), np.asarray(b2, np.float32))

    import time as _t
    key = (tuple(block_buckets), K)
    if key not in _CACHE:
        t0 = _t.time()
        print(f"[kernel] building program nb={len(block_buckets)}", flush=True)
        _CACHE[key] = build_program(block_buckets, K, num_devices=NCORES)
        print(f"[kernel] build done {_t.time()-t0:.1f}s", flush=True)
    nc = _CACHE[key]

    t0 = _t.time()
    print("[kernel] launching on 8 cores", flush=True)
    r = run_bass_kernel_spmd(nc, in_maps, core_ids=list(range(NCORES)),
                             trace=_want_trace)
    print(f"[kernel] run done {_t.time()-t0:.1f}s", flush=True)
    n = pts.shape[0]
    full = np.empty((n, 64), np.float32)
    for c in range(NCORES):
        dev = np.asarray(r.results[c]["out"])
        perm = perms[c]
        valid = perm >= 0
        full[perm[valid]] = dev[valid]
    if _want_trace:
        return full, r
    return full
